# revision 1
# baseline (speedup 1.0000x reference)
"""Complex-valued causal attention head on 8 Trainium2 NeuronCores.

Math (per batch element, fp32 reference):
    q/k/v = complex_linear(x, W*)          # contract C=1024 -> H=64
    wr + i*wi = q @ conj(k)^T              # contract H
    mag = sqrt(wr^2 + wi^2 + 1e-4) / sqrt(H)
    wei = softmax(causal_mask(mag))
    out = wei @ v   (real and imag parts separately)

Sharding: data-parallel over batch B=8 -> one batch element per core, weights
replicated, no collectives. Host-side prep is layout-only: x is passed
pre-transposed (C, T) so the contraction dim lands on partitions, and the six
weight matrices are packed into +-stacked pairs.

Per-core dataflow (T=2048, C=1024, H=64, matmuls in float32r):
  - Complex projections: pre-stacked weight pairs [Wr|Wi] / [-Wi|Wr] let PSUM
    accumulation do all the complex combines; outputs come out H-stacked and
    transposed: K+=[kr;ki], Q+=[qr;qi], Q2=[-qi;qr], V+=[vr;vi], each
    [128, T-chunk].
  - Scores computed TRANSPOSED [tk, tq] so the probabilities are already the
    p^T operand that the PV matmul and the ones-matmul row-sum need:
        psRe = K+[:,tk]^T @ Q+   (= wei_real^T block)
        psIm = K+[:,tk]^T @ Q2   (= -wei_imag^T block; sign dies in squaring)
  - mag^2: re^2 via ACT Square (PSUM->SBUF), im^2 via DVE copy+mult (PSUM has
    a single read port per engine, so tensor_tensor(ps,ps) is illegal);
    GPSIMD adds. Then, quad-coalesced over 4 tk-blocks on ACT:
        p = exp(exp(0.5*ln(s+eps) + ln(H^-0.5)))
    (square/ln/exp share one ACT table set -> no table reloads).
  - Causal mask on diagonal blocks via in-place gpsimd affine_select (p:=0).
  - Row sums via ones-matmul on PE (M=1); PV accumulates out^T [h2, tq] with
    lhsT = V in natural layout (from PE-transposing V+).
  - out^T is PE-transposed back to natural [t, h2]; the row 1/sum scaling
    rides the PSUM->SBUF copy via tensor_scalar_mul; DMA out.
"""

import numpy as np

B, T, C, H = 8, 2048, 1024, 64
H2 = 2 * H            # stacked real|imag head dim = 128
P = 128               # partitions
NCHUNK = 4            # T / 512
CH = T // NCHUNK      # 512 tq columns per chunk
TB = T // P           # 16 tk blocks
EPS = 1e-4
C_SCALE = float(H) ** -0.5
QUAD = 2 * CH         # elementwise coalescing width (2 tk-blocks)

_BUILT = None


def _build(loop_n=None):
    import contextlib

    import concourse.bass as bass
    import concourse.mybir as mybir
    import concourse.tile as tile

    f32 = mybir.dt.float32
    f32r = mybir.dt.float32r
    AF = mybir.ActivationFunctionType

    nc = bass.Bass(trn_type="TRN2")

    # x pre-transposed AND partition-major: [chunk, p, cc, t] so each
    # partition reads one contiguous run per chunk DMA (descriptor-bound
    # otherwise: 2KB strided chunks measured ~37 GB/s, contiguous is fast)
    xr_d = nc.dram_tensor("xT_real", [NCHUNK, P, C // P, CH], f32r, kind="ExternalInput").ap()
    xi_d = nc.dram_tensor("xT_imag", [NCHUNK, P, C // P, CH], f32r, kind="ExternalInput").ap()
    # 7 host-stacked weight blocks, each (C, H2):
    # 0:S1q=[Wqr|Wqi] 1:S2q=[-Wqi|Wqr] 2:S1qn=-S1q 3:S1k 4:S2k 5:S1v 6:S2v
    wst_d = nc.dram_tensor("wstacks", [P, 7, C // P, H2], f32r, kind="ExternalInput").ap()
    # consts: [:, :128]=eye(128), [:, 128]=ones
    consts_d = nc.dram_tensor("consts", [P, P + 1], f32r, kind="ExternalInput").ap()

    # packed output [chunk, p, tb, h2]; host unpacks to (T, H) r/i halves
    out_d = nc.dram_tensor("out_pk", [NCHUNK, P, 4, H2], f32, kind="ExternalOutput").ap()

    CC = C // P  # 8 contraction chunks

    with tile.TileContext(nc) as tc:
        ctx = contextlib.ExitStack()
        with ctx:
            if loop_n is not None:
                ctx.enter_context(tc.For_i(0, loop_n, 1))
            singles = ctx.enter_context(tc.tile_pool(name="singles", bufs=1))
            xt_p = ctx.enter_context(tc.tile_pool(name="xt", bufs=2))
            qc_p = ctx.enter_context(tc.tile_pool(name="qc", bufs=2))
            elw_p = ctx.enter_context(tc.tile_pool(name="elw", bufs=3))
            im_p = ctx.enter_context(tc.tile_pool(name="imp", bufs=3))
            fin_p = ctx.enter_context(tc.tile_pool(name="fin", bufs=2))

            # PSUM budget is 8 banks (2KB/partition each), reserved statically:
            # projps 1 + scps 3 + accps(out+sums) 2 + finps 2 = 8
            projps = ctx.enter_context(tc.tile_pool(name="projps", bufs=1, space="PSUM"))
            scps = ctx.enter_context(tc.tile_pool(name="scps", bufs=3, space="PSUM"))
            accps = ctx.enter_context(tc.tile_pool(name="accps", bufs=1, space="PSUM"))
            finps = ctx.enter_context(tc.tile_pool(name="finps", bufs=1, space="PSUM"))

            # ---- constants ----
            consts_t = singles.tile([P, P + 1], f32r)
            nc.sync.dma_start(consts_t, consts_d)
            ident = consts_t[:, 0:P]
            ones_col = consts_t[:, P:P + 1]
            one1_f32 = consts_t[0:1, P:P + 1].bitcast(f32)

            bias_eps = singles.tile([P, 1], f32)
            nc.vector.memset(bias_eps, EPS)
            bias_lnc = singles.tile([P, 1], f32)
            nc.vector.memset(bias_lnc, float(np.log(C_SCALE)))
            bias_zero = singles.tile([P, 1], f32)
            nc.vector.memset(bias_zero, 0.0)

            wst = singles.tile([P, 7, CC, H2], f32r)
            nc.sync.dma_start(wst, wst_d)
            S1q, S2q, S1qn, S1k, S2k, S1v, S2v = (wst[:, i] for i in range(7))

            # ---- persistent per-batch buffers ----
            k_all = singles.tile([P, T], f32r)       # K+ = [kr^T; ki^T]
            v_nat = singles.tile([P, TB, H2], f32r)  # V natural [t, h2] blocks

            for j in range(NCHUNK):
                # ---------- load xT chunk (pre-transposed on host) ----------
                xt_r = xt_p.tile([P, CC, CH], f32r, tag="xtr")
                xt_i = xt_p.tile([P, CC, CH], f32r, tag="xti")
                c0, c1 = j * CH, (j + 1) * CH
                for h in range(2):
                    cs = slice(4 * h, 4 * h + 4)
                    nc.sync.dma_start(xt_r[:, cs], xr_d[j][:, cs])
                    nc.scalar.dma_start(xt_i[:, cs], xi_d[j][:, cs])

                # ---------- projections (PSUM-accumulated complex) ----------
                def proj(stack_r, stack_i):
                    ps = projps.tile([P, CH], f32, tag="projps")
                    for cc in range(CC):
                        nc.tensor.matmul(ps, stack_r[:, cc], xt_r[:, cc],
                                         start=(cc == 0), stop=False)
                    for cc in range(CC):
                        nc.tensor.matmul(ps, stack_i[:, cc], xt_i[:, cc],
                                         start=False, stop=(cc == CC - 1))
                    return ps

                ps_k = proj(S1k, S2k)
                nc.vector.tensor_copy(k_all[:, c0:c1], ps_k)

                ps_q = proj(S1q, S2q)
                q_c = qc_p.tile([P, CH], f32r, tag="qc")
                nc.scalar.copy(q_c, ps_q)

                ps_q2 = proj(S2q, S1qn)
                q2_c = qc_p.tile([P, CH], f32r, tag="q2c")
                nc.vector.tensor_copy(q2_c, ps_q2)

                ps_v = proj(S1v, S2v)
                vt_c = qc_p.tile([P, CH], f32r, tag="vtc")
                nc.scalar.copy(vt_c, ps_v)
                # V+ [h2, t] -> natural [t, h2] blocks
                ps_vn = finps.tile([P, 4, H2], f32r, tag="vno")
                for t4 in range(4):
                    nc.tensor.transpose(
                        ps_vn[:, t4], vt_c[:, t4 * P:(t4 + 1) * P], ident)
                nc.vector.tensor_copy(v_nat[:, j * 4:(j + 1) * 4], ps_vn)

                # ---------- scores / softmax / PV over tk blocks ----------
                ps_out = accps.tile([P, CH], f32, tag="outps")
                ps_sums = accps.tile([1, CH], f32, tag="sumps")
                nblk = 4 * (j + 1)
                for qd in range(nblk // 2):
                    sq1 = elw_p.tile([P, QUAD], f32, tag="sq1")
                    sq2 = elw_p.tile([P, QUAD], f32, tag="sq2")
                    for b4 in range(2):
                        i = qd * 2 + b4
                        kT = k_all[:, i * P:(i + 1) * P]
                        ps_re = scps.tile([P, CH], f32, tag="sc")
                        nc.tensor.matmul(ps_re, kT, q_c, start=True, stop=True)
                        ps_im = scps.tile([P, CH], f32, tag="sc")
                        nc.tensor.matmul(ps_im, kT, q2_c, start=True, stop=True)
                        # re^2 on ACT (single-PSUM-operand rule)
                        nc.scalar.activation(
                            sq1[:, b4 * CH:(b4 + 1) * CH], ps_re,
                            AF.Square, bias=bias_zero, scale=1.0)
                        # im^2 on DVE: copy out of PSUM, then square in SBUF
                        im_s = im_p.tile([P, CH], f32, tag="ims")
                        nc.vector.tensor_copy(im_s, ps_im)
                        nc.vector.tensor_tensor(
                            sq2[:, b4 * CH:(b4 + 1) * CH], im_s, im_s,
                            mybir.AluOpType.mult)

                    s_t = elw_p.tile([P, QUAD], f32, tag="st")
                    nc.gpsimd.tensor_add(s_t, sq1, sq2)
                    # ln and first exp run in place over s_t
                    nc.scalar.activation(s_t, s_t, AF.Ln,
                                         bias=bias_eps, scale=1.0)
                    nc.scalar.activation(s_t, s_t, AF.Exp,
                                         bias=bias_lnc, scale=0.5)
                    p_t = elw_p.tile([P, QUAD], f32r, tag="pt")
                    nc.scalar.activation(p_t, s_t, AF.Exp,
                                         bias=bias_zero, scale=1.0)

                    for b4 in range(2):
                        i = qd * 2 + b4
                        p_blk = p_t[:, b4 * CH:(b4 + 1) * CH]
                        if i >= 4 * j:  # diagonal: zero where tq < tk
                            nc.gpsimd.affine_select(
                                out=p_blk, in_=p_blk,
                                compare_op=mybir.AluOpType.is_ge,
                                fill=0.0,
                                base=j * CH - i * P,
                                pattern=[[1, CH]],
                                channel_multiplier=-1)
                        nc.tensor.matmul(ps_sums, ones_col, p_blk,
                                         start=(i == 0), stop=(i == nblk - 1))
                        nc.tensor.matmul(ps_out, v_nat[:, i], p_blk,
                                         start=(i == 0), stop=(i == nblk - 1))

                # ---------- finalize chunk ----------
                outT = fin_p.tile([P, CH], f32r, tag="outT")
                nc.vector.tensor_copy(outT, ps_out)
                sums_sb = fin_p.tile([1, CH], f32, tag="sums")
                nc.vector.tensor_copy(sums_sb, ps_sums)

                ps_on = finps.tile([P, 4, H2], f32r, tag="vno")
                for t4 in range(4):
                    nc.tensor.transpose(
                        ps_on[:, t4], outT[:, t4 * P:(t4 + 1) * P], ident)
                ps_rs = finps.tile([P, 4], f32, tag="rsps")
                for t4 in range(4):
                    nc.tensor.matmul(ps_rs[:, t4:t4 + 1],
                                     sums_sb[0:1, t4 * P:(t4 + 1) * P],
                                     one1_f32, start=True, stop=True)
                recip = fin_p.tile([P, 4], f32, tag="recip")
                nc.vector.reciprocal(recip, ps_rs)

                onat = fin_p.tile([P, 4, H2], f32, tag="onat")
                for t4 in range(4):
                    nc.vector.tensor_scalar_mul(
                        onat[:, t4], ps_on[:, t4], recip[:, t4:t4 + 1])
                nc.sync.dma_start(out_d[j], onat)

    _split_multiwaits(nc)
    return nc


def _split_multiwaits(nc):
    """This toolchain's walrus accepts at most ONE sync-wait per instruction;
    Tile's sem-assignment can attach several. Hoist all-but-one wait onto
    standalone InstEventSemaphore carriers (what bass's wait_ge emits)."""
    import concourse.mybir as mybir

    n_split = 0
    for f in nc.m.functions:
        for bb in f.blocks:
            out = []
            for inst in bb.instructions:
                si = inst.sync_info
                if si is not None and si.on_wait and len(si.on_wait) > 1:
                    waits = list(si.on_wait)
                    for w in waits[:-1]:
                        carrier = mybir.InstEventSemaphore(
                            name=f"{inst.name}_wsplit{n_split}", ins=[], outs=[])
                        carrier.engine = inst.engine
                        carrier.sync_info = mybir.SyncInfo(
                            on_wait=[w], on_update=[])
                        out.append(carrier)
                        n_split += 1
                    inst.sync_info = mybir.SyncInfo(
                        on_wait=[waits[-1]], on_update=list(si.on_update))
                out.append(inst)
            bb.instructions = out
    return n_split


def _host_prep(Wk_r, Wk_i, Wq_r, Wq_i, Wv_r, Wv_i):
    def s1(wr, wi):
        return np.concatenate([wr, wi], axis=1)

    def s2(wr, wi):
        return np.concatenate([-wi, wr], axis=1)

    s1q = s1(Wq_r, Wq_i)
    wst = np.stack([
        s1q, s2(Wq_r, Wq_i), -s1q,
        s1(Wk_r, Wk_i), s2(Wk_r, Wk_i),
        s1(Wv_r, Wv_i), s2(Wv_r, Wv_i),
    ]).astype(np.float32)
    # partition-major: (7, C, H2) -> (p, s, cc, h2)
    wst = np.ascontiguousarray(
        wst.reshape(7, C // P, P, H2).transpose(2, 0, 1, 3))
    consts = np.ascontiguousarray(np.concatenate(
        [np.eye(P, dtype=np.float32), np.ones((P, 1), np.float32)], axis=1))
    return wst, consts


def kernel(x_real, x_imag, Wk_r, Wk_i, Wq_r, Wq_i, Wv_r, Wv_i, _trace=False):
    global _BUILT
    from concourse.bass_utils import run_bass_kernel_spmd

    if _BUILT is None:
        _BUILT = _build()
    nc = _BUILT

    wst, consts = _host_prep(
        np.asarray(Wk_r), np.asarray(Wk_i), np.asarray(Wq_r),
        np.asarray(Wq_i), np.asarray(Wv_r), np.asarray(Wv_i))
    x_real = np.asarray(x_real, dtype=np.float32)
    x_imag = np.asarray(x_imag, dtype=np.float32)

    def xprep(xb):
        # (T, C) -> xT (C, T) -> [chunk, p, cc, t] partition-major
        return np.ascontiguousarray(
            xb.T.reshape(C // P, P, NCHUNK, CH).transpose(2, 1, 0, 3))

    in_maps = [
        {
            "xT_real": xprep(x_real[b]),
            "xT_imag": xprep(x_imag[b]),
            "wstacks": wst,
            "consts": consts,
        }
        for b in range(B)
    ]
    res = run_bass_kernel_spmd(nc, in_maps, core_ids=list(range(B)),
                               trace=_trace)
    def unpack(pk):
        # [chunk, p, tb, h2] -> (T, H2)
        full = pk.transpose(0, 2, 1, 3).reshape(T, H2)
        return full[:, 0:H], full[:, H:H2]

    outs = [unpack(res.results[b]["out_pk"]) for b in range(B)]
    out_r = np.ascontiguousarray(np.stack([o[0] for o in outs]))
    out_i = np.ascontiguousarray(np.stack([o[1] for o in outs]))
    if _trace:
        kernel._last_results = res
    return out_r, out_i



# revision 4
# speedup vs baseline: 1.1802x; 1.1802x over previous
"""Complex-valued causal attention head on 8 Trainium2 NeuronCores.

Math (per batch element, fp32 reference):
    q/k/v = complex_linear(x, W*)          # contract C=1024 -> H=64
    wr + i*wi = q @ conj(k)^T              # contract H
    mag = sqrt(wr^2 + wi^2 + 1e-4) / sqrt(H)
    wei = softmax(causal_mask(mag))
    out = wei @ v   (real and imag parts separately)

Sharding: data-parallel over batch B=8 -> one batch element per core, weights
replicated, no collectives. Host-side prep is layout-only + bf16 cast: x is
passed pre-transposed (C, T) so the contraction dim lands on partitions, and
the weight matrices are packed into +-stacked pairs.

Per-core dataflow (T=2048, C=1024, H=64):
  - All matmul operands are bf16 (PSUM accumulates fp32). fp32r moving
    operands stream at ~0.83 ns/row on the PE; bf16 streams at ~0.42 ns/row,
    so this halves the tensor-engine critical path.
  - Complex projections: pre-stacked weight pairs [Wr|Wi] / [-Wi|Wr] let PSUM
    accumulation do all the complex combines; outputs come out H-stacked and
    transposed: K+=[kr;ki], Q+=[qr;qi], V+=[vr;vi], each [128, T-chunk].
  - Q2=[-qi;qr] (the second score operand) is NOT re-projected from x
    (16 matmuls); it's derived from Q+ with a single 128x128 signed
    permutation matmul S.T @ Q+.
  - Scores computed TRANSPOSED [tk, tq] so the probabilities are already the
    p^T operand that the PV matmul and the ones-matmul row-sum need:
        psRe = K+[:,tk]^T @ Q+   (= wei_real^T block)
        psIm = K+[:,tk]^T @ Q2   (= -wei_imag^T block; sign dies in squaring)
  - mag^2: re^2 via ACT Square (PSUM->SBUF, bf16 out), im^2 via DVE copy+mult
    (PSUM has a single read port per engine, so tensor_tensor(ps,ps) is
    illegal); a fraction of the re exits also go via DVE to balance ACT/DVE
    load. GPSIMD adds. Then, quad-coalesced over 2 tk-blocks on ACT:
        p = exp(exp(0.5*ln(s+eps) + ln(H^-0.5)))
    (square/ln/exp share one ACT table set -> no table reloads).
  - Causal mask on diagonal blocks via in-place gpsimd affine_select (p:=0).
  - Row sums via ones-matmul on PE (M=1); PV accumulates out^T [h2, tq] with
    lhsT = V in natural layout (from PE-transposing V+). The sums/PV matmuls
    for group g are emitted after the scores of group g+LAG so the PE never
    waits on the elementwise chain.
  - out^T is PE-transposed back to natural [t, h2]; the row 1/sum scaling
    rides the PSUM->SBUF copy via tensor_scalar_mul; DMA out.
"""

import numpy as np

B, T, C, H = 8, 2048, 1024, 64
H2 = 2 * H            # stacked real|imag head dim = 128
P = 128               # partitions
NCHUNK = 4            # T / 512
CH = T // NCHUNK      # 512 tq columns per chunk
TB = T // P           # 16 tk blocks
EPS = 1e-4
C_SCALE = float(H) ** -0.5
QUAD = 2 * CH         # elementwise coalescing width (2 tk-blocks)
LAG = 2               # groups of score->softmax in flight before PV emission
ACT_EXIT_NUM, ACT_EXIT_DEN = 3, 5   # fraction of re^2 exits taken by ACT

_BUILT = None


def _build(loop_n=None):
    import contextlib

    import concourse.bass as bass
    import concourse.mybir as mybir
    import concourse.tile as tile

    f32 = mybir.dt.float32
    bf16 = mybir.dt.bfloat16
    AF = mybir.ActivationFunctionType
    ALU = mybir.AluOpType

    nc = bass.Bass(trn_type="TRN2")

    # x pre-transposed AND partition-major: [chunk, p, cc, t] so each
    # partition reads one contiguous run per chunk DMA
    xr_d = nc.dram_tensor("xT_real", [NCHUNK, P, C // P, CH], bf16, kind="ExternalInput").ap()
    xi_d = nc.dram_tensor("xT_imag", [NCHUNK, P, C // P, CH], bf16, kind="ExternalInput").ap()
    # 6 host-stacked weight blocks, each (C, H2):
    # 0:S1q=[Wqr|Wqi] 1:S2q=[-Wqi|Wqr] 2:S1k 3:S2k 4:S1v 5:S2v
    wst_d = nc.dram_tensor("wstacks", [P, 6, C // P, H2], bf16, kind="ExternalInput").ap()
    # consts: [:, :128]=eye(128), [:, 128:256]=S_T perm, [:, 256]=ones
    consts_d = nc.dram_tensor("consts", [P, 2 * P + 1], bf16, kind="ExternalInput").ap()

    # packed output [chunk, p, tb, h2]; host unpacks to (T, H) r/i halves
    out_d = nc.dram_tensor("out_pk", [NCHUNK, P, 4, H2], f32, kind="ExternalOutput").ap()

    CC = C // P  # 8 contraction chunks

    with tile.TileContext(nc) as tc:
        ctx = contextlib.ExitStack()
        with ctx:
            if loop_n is not None:
                ctx.enter_context(tc.For_i(0, loop_n, 1))
            singles = ctx.enter_context(tc.tile_pool(name="singles", bufs=1))
            xt_p = ctx.enter_context(tc.tile_pool(name="xt", bufs=2))
            qc_p = ctx.enter_context(tc.tile_pool(name="qc", bufs=2))
            elw_p = ctx.enter_context(tc.tile_pool(name="elw", bufs=3))
            im_p = ctx.enter_context(tc.tile_pool(name="imp", bufs=3))
            p_p = ctx.enter_context(tc.tile_pool(name="pp", bufs=LAG + 2))
            fin_p = ctx.enter_context(tc.tile_pool(name="fin", bufs=2))

            # PSUM budget is 8 banks (2KB/partition each), reserved statically:
            # projps 1 + scps 3 + accps(out+sums) 2 + finps 2 = 8
            projps = ctx.enter_context(tc.tile_pool(name="projps", bufs=1, space="PSUM"))
            scps = ctx.enter_context(tc.tile_pool(name="scps", bufs=3, space="PSUM"))
            accps = ctx.enter_context(tc.tile_pool(name="accps", bufs=1, space="PSUM"))
            finps = ctx.enter_context(tc.tile_pool(name="finps", bufs=1, space="PSUM"))

            # ---- constants ----
            consts_t = singles.tile([P, 2 * P + 1], bf16)
            nc.sync.dma_start(consts_t, consts_d)
            ident = consts_t[:, 0:P]
            s_perm = consts_t[:, P:2 * P]
            ones_col = consts_t[:, 2 * P:2 * P + 1]

            one1_f32 = singles.tile([1, 1], f32)
            nc.vector.memset(one1_f32, 1.0)

            bias_eps = singles.tile([P, 1], f32)
            nc.vector.memset(bias_eps, EPS)
            bias_lnc = singles.tile([P, 1], f32)
            nc.vector.memset(bias_lnc, float(np.log(C_SCALE)))
            bias_zero = singles.tile([P, 1], f32)
            nc.vector.memset(bias_zero, 0.0)

            wst = singles.tile([P, 6, CC, H2], bf16)
            nc.sync.dma_start(wst, wst_d)
            S1q, S2q, S1k, S2k, S1v, S2v = (wst[:, i] for i in range(6))

            # ---- persistent per-batch buffers ----
            k_all = singles.tile([P, T], bf16)       # K+ = [kr^T; ki^T]
            v_nat = singles.tile([P, TB, H2], bf16)  # V natural [t, h2] blocks

            exit_ctr = [0]  # global counter for ACT/DVE re^2 exit split

            for j in range(NCHUNK):
                # ---------- load xT chunk (pre-transposed on host) ----------
                xt_r = xt_p.tile([P, CC, CH], bf16, tag="xtr")
                xt_i = xt_p.tile([P, CC, CH], bf16, tag="xti")
                for h in range(2):
                    cs = slice(4 * h, 4 * h + 4)
                    nc.sync.dma_start(xt_r[:, cs], xr_d[j][:, cs])
                    nc.scalar.dma_start(xt_i[:, cs], xi_d[j][:, cs])

                # ---------- projections (PSUM-accumulated complex) ----------
                def proj(stack_r, stack_i):
                    ps = projps.tile([P, CH], f32, tag="projps")
                    for cc in range(CC):
                        nc.tensor.matmul(ps, stack_r[:, cc], xt_r[:, cc],
                                         start=(cc == 0), stop=False)
                    for cc in range(CC):
                        nc.tensor.matmul(ps, stack_i[:, cc], xt_i[:, cc],
                                         start=False, stop=(cc == CC - 1))
                    return ps

                c0, c1 = j * CH, (j + 1) * CH
                ps_k = proj(S1k, S2k)
                nc.vector.tensor_copy(k_all[:, c0:c1], ps_k)

                ps_q = proj(S1q, S2q)
                q_c = qc_p.tile([P, CH], bf16, tag="qc")
                nc.scalar.copy(q_c, ps_q)

                # Q2 = [-qi; qr] = S @ Q+ via one signed-permutation matmul
                ps_q2 = scps.tile([P, CH], f32, tag="sc")
                nc.tensor.matmul(ps_q2, s_perm, q_c, start=True, stop=True)
                q2_c = qc_p.tile([P, CH], bf16, tag="q2c")
                nc.vector.tensor_copy(q2_c, ps_q2)

                ps_v = proj(S1v, S2v)
                vt_c = qc_p.tile([P, CH], bf16, tag="vtc")
                nc.vector.tensor_copy(vt_c, ps_v)
                # V+ [h2, t] -> natural [t, h2] blocks
                ps_vn = finps.tile([P, 4, H2], bf16, tag="vno")
                for t4 in range(4):
                    nc.tensor.transpose(
                        ps_vn[:, t4], vt_c[:, t4 * P:(t4 + 1) * P], ident)
                nc.vector.tensor_copy(v_nat[:, j * 4:(j + 1) * 4], ps_vn)

                # ---------- scores / softmax / PV over tk blocks ----------
                ps_out = accps.tile([P, CH], f32, tag="outps")
                ps_sums = accps.tile([1, CH], f32, tag="sumps")
                nblk = 4 * (j + 1)
                ngrp = nblk // 2

                def stage_a(qd):
                    """scores + exits + squares + add + ln/exp/exp + mask for
                    the 2-block group qd. Returns the p tile."""
                    sq1 = elw_p.tile([P, QUAD], bf16, tag="sq1")
                    sq2 = elw_p.tile([P, QUAD], bf16, tag="sq2")
                    for b4 in range(2):
                        i = qd * 2 + b4
                        kT = k_all[:, i * P:(i + 1) * P]
                        ps_re = scps.tile([P, CH], f32, tag="sc")
                        nc.tensor.matmul(ps_re, kT, q_c, start=True, stop=True)
                        ps_im = scps.tile([P, CH], f32, tag="sc")
                        nc.tensor.matmul(ps_im, kT, q2_c, start=True, stop=True)
                        sq1_blk = sq1[:, b4 * CH:(b4 + 1) * CH]
                        # re^2: split between ACT (fused square, reads PSUM)
                        # and DVE (copy+mult) to balance engine load
                        if (exit_ctr[0] % ACT_EXIT_DEN) < ACT_EXIT_NUM:
                            nc.scalar.activation(
                                sq1_blk, ps_re, AF.Square,
                                bias=bias_zero, scale=1.0)
                        else:
                            re_s = im_p.tile([P, CH], bf16, tag="res")
                            nc.vector.tensor_copy(re_s, ps_re)
                            nc.vector.tensor_tensor(
                                sq1_blk, re_s, re_s, ALU.mult)
                        exit_ctr[0] += 1
                        # im^2 on DVE: copy out of PSUM, then square in SBUF
                        im_s = im_p.tile([P, CH], bf16, tag="ims")
                        nc.vector.tensor_copy(im_s, ps_im)
                        nc.vector.tensor_tensor(
                            sq2[:, b4 * CH:(b4 + 1) * CH], im_s, im_s,
                            ALU.mult)

                    s_t = elw_p.tile([P, QUAD], bf16, tag="st")
                    nc.gpsimd.tensor_tensor(s_t, sq1, sq2, ALU.add)
                    m_t = elw_p.tile([P, QUAD], f32, tag="mt")
                    # ln and first exp run in place over m_t
                    nc.scalar.activation(m_t, s_t, AF.Ln,
                                         bias=bias_eps, scale=1.0)
                    nc.scalar.activation(m_t, m_t, AF.Exp,
                                         bias=bias_lnc, scale=0.5)
                    p_t = p_p.tile([P, QUAD], bf16, tag="pt")
                    nc.scalar.activation(p_t, m_t, AF.Exp,
                                         bias=bias_zero, scale=1.0)

                    for b4 in range(2):
                        i = qd * 2 + b4
                        if i >= 4 * j:  # diagonal: zero where tq < tk
                            p_blk = p_t[:, b4 * CH:(b4 + 1) * CH]
                            nc.gpsimd.affine_select(
                                out=p_blk, in_=p_blk,
                                compare_op=ALU.is_ge,
                                fill=0.0,
                                base=j * CH - i * P,
                                pattern=[[1, CH]],
                                channel_multiplier=-1)
                    return p_t

                def stage_b(qd, p_t):
                    """row-sum + PV accumulation matmuls for group qd."""
                    for b4 in range(2):
                        i = qd * 2 + b4
                        p_blk = p_t[:, b4 * CH:(b4 + 1) * CH]
                        nc.tensor.matmul(ps_sums, ones_col, p_blk,
                                         start=(i == 0), stop=(i == nblk - 1))
                        nc.tensor.matmul(ps_out, v_nat[:, i], p_blk,
                                         start=(i == 0), stop=(i == nblk - 1))

                pending = []
                for qd in range(ngrp):
                    p_t = stage_a(qd)
                    pending.append((qd, p_t))
                    if len(pending) > LAG:
                        stage_b(*pending.pop(0))
                while pending:
                    stage_b(*pending.pop(0))

                # ---------- finalize chunk ----------
                outT = fin_p.tile([P, CH], bf16, tag="outT")
                nc.vector.tensor_copy(outT, ps_out)
                sums_sb = fin_p.tile([1, CH], f32, tag="sums")
                nc.vector.tensor_copy(sums_sb, ps_sums)

                ps_on = finps.tile([P, 4, H2], bf16, tag="vno")
                for t4 in range(4):
                    nc.tensor.transpose(
                        ps_on[:, t4], outT[:, t4 * P:(t4 + 1) * P], ident)
                ps_rs = finps.tile([P, 4], f32, tag="rsps")
                for t4 in range(4):
                    nc.tensor.matmul(ps_rs[:, t4:t4 + 1],
                                     sums_sb[0:1, t4 * P:(t4 + 1) * P],
                                     one1_f32, start=True, stop=True)
                recip = fin_p.tile([P, 4], f32, tag="recip")
                nc.vector.reciprocal(recip, ps_rs)

                onat = fin_p.tile([P, 4, H2], f32, tag="onat")
                for t4 in range(4):
                    nc.vector.tensor_scalar_mul(
                        onat[:, t4], ps_on[:, t4], recip[:, t4:t4 + 1])
                nc.sync.dma_start(out_d[j], onat)

    _split_multiwaits(nc)
    return nc


def _split_multiwaits(nc):
    """This toolchain's walrus accepts at most ONE sync-wait per instruction;
    Tile's sem-assignment can attach several. Hoist all-but-one wait onto
    standalone InstEventSemaphore carriers (what bass's wait_ge emits)."""
    import concourse.mybir as mybir

    n_split = 0
    for f in nc.m.functions:
        for bb in f.blocks:
            out = []
            for inst in bb.instructions:
                si = inst.sync_info
                if si is not None and si.on_wait and len(si.on_wait) > 1:
                    waits = list(si.on_wait)
                    for w in waits[:-1]:
                        carrier = mybir.InstEventSemaphore(
                            name=f"{inst.name}_wsplit{n_split}", ins=[], outs=[])
                        carrier.engine = inst.engine
                        carrier.sync_info = mybir.SyncInfo(
                            on_wait=[w], on_update=[])
                        out.append(carrier)
                        n_split += 1
                    inst.sync_info = mybir.SyncInfo(
                        on_wait=[waits[-1]], on_update=list(si.on_update))
                out.append(inst)
            bb.instructions = out
    return n_split


def _host_prep(Wk_r, Wk_i, Wq_r, Wq_i, Wv_r, Wv_i):
    import ml_dtypes

    bf16 = ml_dtypes.bfloat16

    def s1(wr, wi):
        return np.concatenate([wr, wi], axis=1)

    def s2(wr, wi):
        return np.concatenate([-wi, wr], axis=1)

    wst = np.stack([
        s1(Wq_r, Wq_i), s2(Wq_r, Wq_i),
        s1(Wk_r, Wk_i), s2(Wk_r, Wk_i),
        s1(Wv_r, Wv_i), s2(Wv_r, Wv_i),
    ]).astype(bf16)
    # partition-major: (6, C, H2) -> (p, s, cc, h2)
    wst = np.ascontiguousarray(
        wst.reshape(6, C // P, P, H2).transpose(2, 0, 1, 3))
    # S with S @ [qr; qi] = [-qi; qr]; matmul computes lhsT.T @ rhs so pass S^T
    s_mat = np.zeros((P, P), np.float32)
    for i in range(H):
        s_mat[i, H + i] = -1.0
        s_mat[H + i, i] = 1.0
    consts = np.ascontiguousarray(np.concatenate(
        [np.eye(P, dtype=np.float32), s_mat.T, np.ones((P, 1), np.float32)],
        axis=1)).astype(bf16)
    return wst, consts


def kernel(x_real, x_imag, Wk_r, Wk_i, Wq_r, Wq_i, Wv_r, Wv_i, _trace=False):
    global _BUILT
    import ml_dtypes
    from concourse.bass_utils import run_bass_kernel_spmd

    bf16 = ml_dtypes.bfloat16

    if _BUILT is None:
        _BUILT = _build()
    nc = _BUILT

    wst, consts = _host_prep(
        np.asarray(Wk_r), np.asarray(Wk_i), np.asarray(Wq_r),
        np.asarray(Wq_i), np.asarray(Wv_r), np.asarray(Wv_i))
    x_real = np.asarray(x_real, dtype=np.float32)
    x_imag = np.asarray(x_imag, dtype=np.float32)

    def xprep(xb):
        # (T, C) -> xT (C, T) -> [chunk, p, cc, t] partition-major, bf16
        return np.ascontiguousarray(
            xb.T.reshape(C // P, P, NCHUNK, CH).transpose(2, 1, 0, 3)
            .astype(bf16))

    in_maps = [
        {
            "xT_real": xprep(x_real[b]),
            "xT_imag": xprep(x_imag[b]),
            "wstacks": wst,
            "consts": consts,
        }
        for b in range(B)
    ]
    res = run_bass_kernel_spmd(nc, in_maps, core_ids=list(range(B)),
                               trace=_trace)
    def unpack(pk):
        # [chunk, p, tb, h2] -> (T, H2)
        full = pk.transpose(0, 2, 1, 3).reshape(T, H2)
        return full[:, 0:H], full[:, H:H2]

    outs = [unpack(res.results[b]["out_pk"]) for b in range(B)]
    out_r = np.ascontiguousarray(np.stack([o[0] for o in outs]))
    out_i = np.ascontiguousarray(np.stack([o[1] for o in outs]))
    if _trace:
        kernel._last_results = res
    return out_r, out_i
